# revision 29
# baseline (speedup 1.0000x reference)
"""XNOR-Net++ 3x3 conv (sign(x) (*) sign(w) * alpha*beta*gamma) on 8 TRN2 NeuronCores.

Sharding: data-parallel over batch (32 -> 4 per core), weights/scales replicated.

Per core:
- x and the pre-transposed weight are staged to HBM as bf16 (sign-preserving cast,
  halves DMA); output is written bf16 and upcast on host (conv values are integers
  <= 2304, bf16 rel err < 0.4% << 2e-2 gate)
- weights arrive pre-transposed from host ([i, ky*kx, ob, cb, o] layout),
  binarized on-device to fp8 in two ACT ops that overlap the split weight DMA
  (no PE transposes)
- sign image: ONE padded fp8 buffer [128, 2, 58, 64] per image (double-buffered,
  borders zeroed once in the preamble); the 9 conv taps read strided windows
  [*, *, t*8+ky : +8, kx : kx+56] directly -- no shifted copies, no per-image
  memsets; sign runs in 7 row-chunks so early row-tiles' matmuls start ASAP
- PE warm-up filler matmuls bridge the DMA-bound startup so the HAM clock gate
  stays at 2.4 GHz when the conv stream begins (cold-start costs ~25us otherwise)
- 3x3 conv = 9 accumulating DoubleRow fp8 matmuls per [128, 448] output tile
  (K=256 via input-channel-block pairing, 2 fp8 weights/PE cell)
- epilogue: single DVE mul with a precomputed alpha*beta*gamma map, built from
  two tiny [1,56] partition-broadcasts + DVE outer-product muls (fast enough
  that ob=0's epilogues can free their PSUM banks before ob=1 needs them)
"""

import os
from contextlib import ExitStack

import ml_dtypes
import numpy as np

import concourse.bacc as bacc
import concourse.mybir as mybir
import concourse.tile as tile
from concourse.bass_utils import run_bass_kernel_spmd

# Without this, a run that follows a crashed/interrupted one can die with
# NRT_EXEC_UNIT_UNRECOVERABLE at device open; the reset is cheap.
os.environ.setdefault("NEURON_RT_RESET_CORES", "1")

N_CORES = 8
B, C, H, KS = 32, 256, 56, 3
P = 128
CB = C // P  # input-channel blocks (2)
OB = C // P  # output-channel blocks (2)
HP = H + 2   # padded image rows (58)
WP = 64      # padded image row pitch (cols 0..57 live, 58..63 never read)
R = 8        # output rows per matmul tile
T = H // R   # row tiles per image (7)
NT = R * H   # moving free dim per matmul (448)
HW = H * H   # pixels per image (3136)
RA = 9       # first x chunk (data rows 0..8) -- lands early, unblocks tile t=0
RT = 24      # second x chunk (data rows 9..32)
RB = H - RA - RT  # third x chunk (data rows 33..55)
# sign row-chunks (data-row ranges); chunk 0 reads xA, 1-3 read xT, 4-6 read xB
CHUNKS = [(0, 9), (9, 17), (17, 25), (25, 33), (33, 41), (41, 49), (49, 56)]
N_WARM = 14  # PE warm-up fillers bridging the DMA-bound startup
WSPLIT = KS * KS * CB * P  # weight half boundary: all of ob=0 | all of ob=1
W0A = 5 * CB * P  # ob=0 sub-split: taps 0-4 land+sign first, t=0 starts early

F32 = mybir.dt.float32
BF16 = mybir.dt.bfloat16
FP8 = mybir.dt.float8e4
DR = mybir.MatmulPerfMode.DoubleRow


def build_conv(tc, out_ap, x_ap, wT_ap, a_ap, b_ap, g_ap, BL):
    nc = tc.nc
    with ExitStack() as ctx:
        const_pool = ctx.enter_context(tc.tile_pool(name="const", bufs=1))
        wpool = ctx.enter_context(tc.tile_pool(name="w", bufs=1))
        xpool = ctx.enter_context(tc.tile_pool(name="x", bufs=2))
        impool = ctx.enter_context(tc.tile_pool(name="img", bufs=1))
        psumpool = ctx.enter_context(tc.tile_pool(name="psum", bufs=4, space="PSUM"))
        opool = ctx.enter_context(tc.tile_pool(name="o", bufs=4))

        # ---- weights first (the startup-critical DMA), in ob-major halves:
        # conv does ob=0 first, so only half the weights gate the full-rate
        # stream; the ob=1 half DMAs+signs under the first 12us of conv ----
        x_v = x_ap.rearrange("b (cb p) h w -> b p cb (h w)", p=P)
        w_bf = wpool.tile([P, KS * KS * OB * CB * P], BF16, name="w_bf")
        nc.sync.dma_start(w_bf[:, 0:W0A], wT_ap[:, 0:W0A])
        # image 0's first row-chunk rides between the weight chunks so the
        # first sign lands as early as possible
        xA0 = xpool.tile([P, CB, RA * H], BF16, name="xA", tag="xA")
        nc.sync.dma_start(xA0, x_v[0][:, :, 0 : RA * H])
        nc.sync.dma_start(w_bf[:, W0A:WSPLIT], wT_ap[:, W0A:WSPLIT])
        nc.sync.dma_start(
            w_bf[:, WSPLIT : KS * KS * OB * CB * P],
            wT_ap[:, WSPLIT : KS * KS * OB * CB * P],
        )
        wT2 = wpool.tile([P, KS * KS * OB * CB * P], FP8, name="wT2")
        nc.scalar.sign(wT2[:, 0:W0A], w_bf[:, 0:W0A])
        # ob=1 half's sign is emitted after image 0's row-chunk signs
        # wv[i_low, ob, tap, cb, o]; pair dim cb has byte-step 128 (%16==0)
        wv = wT2.rearrange("p (ob kk cb o) -> p ob kk cb o", kk=KS * KS, ob=OB, cb=CB)

        # ---- PE warm-up fillers: keep HAM at 2.4 GHz until the conv stream ----
        ones_t = const_pool.tile([1, NT], BF16, name="ones_t")
        nc.gpsimd.memset(ones_t, 1.0)
        warm_ps = psumpool.tile([P, NT], F32, name="warm", tag="warm", bufs=1)
        for _ in range(N_WARM):
            nc.tensor.matmul(
                warm_ps, ones_t[0:1, 0:P], ones_t, start=True, stop=True
            )

        # ---- persistent padded sign images; borders zeroed once ----
        ims = [
            impool.tile([P, CB, HP, WP], FP8, name=f"im{i}", tag=f"im{i}")
            for i in range(2)
        ]
        for im in ims:
            nc.gpsimd.memset(im[:, :, 0, 0:58], 0.0)
            nc.gpsimd.memset(im[:, :, HP - 1, 0:58], 0.0)
            nc.gpsimd.memset(im[:, :, 1 : HP - 1, 0], 0.0)
            nc.gpsimd.memset(im[:, :, 1 : HP - 1, 57], 0.0)

        abg = const_pool.tile([P, OB, HW], BF16, name="abg")

        # ---- main loop over local batches ----
        out_v = out_ap.rearrange("b (ob p) h w -> b ob p (h w)", p=P)
        for b in range(BL):
            im = ims[b % 2]
            if b == 0:
                xA = xA0
            else:
                xA = xpool.tile([P, CB, RA * H], BF16, name="xA", tag="xA")
                nc.sync.dma_start(xA, x_v[b][:, :, 0 : RA * H])
            xT = xpool.tile([P, CB, RT * H], BF16, name="xT", tag="xT")
            nc.sync.dma_start(xT, x_v[b][:, :, RA * H : (RA + RT) * H])
            xB = xpool.tile([P, CB, RB * H], BF16, name="xB", tag="xB")
            nc.sync.dma_start(xB, x_v[b][:, :, (RA + RT) * H : HW])
            views = [
                (0, xA.rearrange("p c (h w) -> p c h w", h=RA)),
                (RA, xT.rearrange("p c (h w) -> p c h w", h=RT)),
                (RA + RT, xB.rearrange("p c (h w) -> p c h w", h=RB)),
            ]
            for ci, (r0, r1) in enumerate(CHUNKS):
                v0, view = next(v for v in reversed(views) if v[0] <= r0)
                nc.scalar.sign(
                    im[:, :, 1 + r0 : 1 + r1, 1 : 1 + H],
                    view[:, :, r0 - v0 : r1 - v0, :],
                )
                if b == 0 and ci == 0:
                    nc.scalar.sign(wT2[:, W0A:WSPLIT], w_bf[:, W0A:WSPLIT])
            if b == 0:
                nc.scalar.sign(
                    wT2[:, WSPLIT : KS * KS * OB * CB * P],
                    w_bf[:, WSPLIT : KS * KS * OB * CB * P],
                )

            if b == 0:
                # tiny scale DMAs + alpha*beta*gamma map. Emitted after the
                # startup-critical dispatches (w, image-0 x) so the Sync engine
                # doesn't delay those. The map must be ready ~12us after the
                # first conv MM (when ob=0's epilogues must free PSUM banks),
                # so it is built from two tiny [1,56] partition-broadcasts and
                # DVE outer-product muls -- a ~5us chain, all small ops.
                a_t = const_pool.tile([P, OB], F32, name="a_t")
                nc.sync.dma_start(
                    a_t, a_ap.rearrange("(ob p) u v -> p (ob u v)", p=P)
                )
                b_t = const_pool.tile([1, H], F32, name="b_t")
                nc.sync.dma_start(b_t, b_ap[0:1, :, 0])
                g_t = const_pool.tile([1, H], F32, name="g_t")
                nc.sync.dma_start(g_t, g_ap[0:1, 0, :])
                b_bc = const_pool.tile([P, H], F32, name="b_bc")
                nc.gpsimd.partition_broadcast(b_bc, b_t)
                g_bc = const_pool.tile([P, H], F32, name="g_bc")
                nc.gpsimd.partition_broadcast(g_bc, g_t)
                ab = const_pool.tile([P, OB, H], F32, name="ab")
                for ob in range(OB):
                    nc.vector.tensor_scalar_mul(
                        ab[:, ob, :], b_bc, a_t[:, ob : ob + 1]
                    )
                    nc.vector.tensor_mul(
                        abg[:, ob, :].rearrange("p (i j) -> p i j", i=H),
                        ab[:, ob, :].unsqueeze(2).to_broadcast((P, H, H)),
                        g_bc.unsqueeze(1).to_broadcast((P, H, H)),
                    )

            for ob in range(OB):
                for t in range(T):
                    ps = psumpool.tile([P, NT], F32, name="cps", tag="cps", bufs=7)
                    for kk in range(KS * KS):
                        ky, kx = divmod(kk, KS)
                        rhs = im[:, :, t * R + ky : t * R + ky + R, kx : kx + H]
                        nc.tensor.matmul(
                            ps,
                            wv[:, ob, kk],
                            rhs,
                            start=(kk == 0),
                            stop=(kk == KS * KS - 1),
                            perf_mode=DR,
                        )
                    sl = slice(t * NT, (t + 1) * NT)
                    ot = opool.tile([P, NT], BF16, name="ot")
                    if b == 0 and ob == 0:
                        # abg may not be ready yet -- copy PSUM out first so
                        # the bank frees for ob=1's matmuls, multiply later
                        tmp = opool.tile([P, NT], F32, name="etmp", tag="etmp")
                        nc.vector.tensor_copy(tmp, ps)
                        nc.vector.tensor_mul(ot, tmp, abg[:, ob, sl])
                    else:
                        nc.vector.tensor_mul(ot, ps, abg[:, ob, sl])
                    nc.sync.dma_start(out_v[b, ob][:, sl], ot)


def build_nc(BL):
    nc = bacc.Bacc("TRN2", target_bir_lowering=False, debug=False)
    x = nc.dram_tensor("x", [BL, C, H, H], BF16, kind="ExternalInput")
    wT = nc.dram_tensor(
        "weightT", [P, KS * KS * OB * CB * P], BF16, kind="ExternalInput"
    )
    a = nc.dram_tensor("alpha", [C, 1, 1], F32, kind="ExternalInput")
    be = nc.dram_tensor("beta", [1, H, 1], F32, kind="ExternalInput")
    g = nc.dram_tensor("gamma", [1, 1, H], F32, kind="ExternalInput")
    o = nc.dram_tensor("out", [BL, C, H, H], BF16, kind="ExternalOutput")
    with tile.TileContext(nc) as tc:
        build_conv(tc, o.ap(), x.ap(), wT.ap(), a.ap(), be.ap(), g.ap(), BL)
    nc.compile()
    return nc


_nc_cache = {}


def _get_nc(BL):
    if BL not in _nc_cache:
        _nc_cache[BL] = build_nc(BL)
    return _nc_cache[BL]


def _prep(x, weight, alpha, beta, gamma):
    """Build the bass kernel and the per-core input maps."""
    x = np.asarray(x, dtype=np.float32)
    weight = np.asarray(weight, dtype=np.float32)
    alpha = np.ascontiguousarray(np.asarray(alpha, dtype=np.float32))
    beta = np.ascontiguousarray(np.asarray(beta, dtype=np.float32))
    gamma = np.ascontiguousarray(np.asarray(gamma, dtype=np.float32))

    # bf16 staging: sign(bf16(v)) == sign(v) for all practically occurring values
    x_bf = np.ascontiguousarray(x.astype(ml_dtypes.bfloat16))
    # [o, i, ky, kx] -> [i_low, ob, (ky kx), cb, o_low]
    w6 = weight.reshape(OB, P, CB, P, KS, KS)
    wT = np.ascontiguousarray(
        w6.transpose(3, 0, 4, 5, 2, 1).astype(ml_dtypes.bfloat16)
    ).reshape(P, KS * KS * OB * CB * P)

    BL = B // N_CORES
    nc = _get_nc(BL)
    xs = x_bf.reshape(N_CORES, BL, C, H, H)
    in_maps = [
        {"x": xs[c], "weightT": wT, "alpha": alpha, "beta": beta, "gamma": gamma}
        for c in range(N_CORES)
    ]
    return nc, in_maps


def kernel(x, weight, alpha, beta, gamma):
    nc, in_maps = _prep(x, weight, alpha, beta, gamma)
    res = run_bass_kernel_spmd(nc, in_maps, list(range(N_CORES)))
    out = np.concatenate([r["out"] for r in res.results], axis=0)
    return out.astype(np.float32)


# revision 30
# speedup vs baseline: 1.0009x; 1.0009x over previous
"""XNOR-Net++ 3x3 conv (sign(x) (*) sign(w) * alpha*beta*gamma) on 8 TRN2 NeuronCores.

Sharding: data-parallel over batch (32 -> 4 per core), weights/scales replicated.

Per core:
- x and the pre-transposed weight are staged to HBM as bf16 (sign-preserving cast,
  halves DMA); output is written bf16 and upcast on host (conv values are integers
  <= 2304, bf16 rel err < 0.4% << 2e-2 gate)
- weights arrive pre-transposed from host ([i, ky*kx, ob, cb, o] layout),
  binarized on-device to fp8 in two ACT ops that overlap the split weight DMA
  (no PE transposes)
- sign image: ONE padded fp8 buffer [128, 2, 58, 64] per image (double-buffered,
  borders zeroed once in the preamble); the 9 conv taps read strided windows
  [*, *, t*8+ky : +8, kx : kx+56] directly -- no shifted copies, no per-image
  memsets; sign runs in 7 row-chunks so early row-tiles' matmuls start ASAP
- PE warm-up filler matmuls bridge the DMA-bound startup so the HAM clock gate
  stays at 2.4 GHz when the conv stream begins (cold-start costs ~25us otherwise)
- 3x3 conv = 9 accumulating DoubleRow fp8 matmuls per [128, 448] output tile
  (K=256 via input-channel-block pairing, 2 fp8 weights/PE cell)
- epilogue: single DVE mul with a precomputed alpha*beta*gamma map, built from
  two tiny [1,56] partition-broadcasts + DVE outer-product muls (fast enough
  that ob=0's epilogues can free their PSUM banks before ob=1 needs them)
"""

import os
from contextlib import ExitStack

import ml_dtypes
import numpy as np

import concourse.bacc as bacc
import concourse.mybir as mybir
import concourse.tile as tile
from concourse.bass_utils import run_bass_kernel_spmd

# Without this, a run that follows a crashed/interrupted one can die with
# NRT_EXEC_UNIT_UNRECOVERABLE at device open; the reset is cheap.
os.environ.setdefault("NEURON_RT_RESET_CORES", "1")

N_CORES = 8
B, C, H, KS = 32, 256, 56, 3
P = 128
CB = C // P  # input-channel blocks (2)
OB = C // P  # output-channel blocks (2)
HP = H + 2   # padded image rows (58)
WP = 64      # padded image row pitch (cols 0..57 live, 58..63 never read)
R = 8        # output rows per matmul tile
T = H // R   # row tiles per image (7)
NT = R * H   # moving free dim per matmul (448)
HW = H * H   # pixels per image (3136)
RA = 9       # first x chunk (data rows 0..8) -- lands early, unblocks tile t=0
RT = 24      # second x chunk (data rows 9..32)
RB = H - RA - RT  # third x chunk (data rows 33..55)
# sign row-chunks (data-row ranges); chunk 0 reads xA, 1-3 read xT, 4-6 read xB
CHUNKS = [(0, 9), (9, 17), (17, 25), (25, 33), (33, 41), (41, 49), (49, 56)]
N_WARM = 20  # PE warm-up fillers bridging the DMA-bound startup
WSPLIT = KS * KS * CB * P  # weight half boundary: all of ob=0 | all of ob=1

F32 = mybir.dt.float32
BF16 = mybir.dt.bfloat16
FP8 = mybir.dt.float8e4
DR = mybir.MatmulPerfMode.DoubleRow


def build_conv(tc, out_ap, x_ap, wT_ap, a_ap, b_ap, g_ap, BL):
    nc = tc.nc
    with ExitStack() as ctx:
        const_pool = ctx.enter_context(tc.tile_pool(name="const", bufs=1))
        wpool = ctx.enter_context(tc.tile_pool(name="w", bufs=1))
        xpool = ctx.enter_context(tc.tile_pool(name="x", bufs=2))
        impool = ctx.enter_context(tc.tile_pool(name="img", bufs=1))
        psumpool = ctx.enter_context(tc.tile_pool(name="psum", bufs=4, space="PSUM"))
        opool = ctx.enter_context(tc.tile_pool(name="o", bufs=4))

        # ---- weights first (the startup-critical DMA), in ob-major halves:
        # conv does ob=0 first, so only half the weights gate the full-rate
        # stream; the ob=1 half DMAs+signs under the first 12us of conv ----
        x_v = x_ap.rearrange("b (cb p) h w -> b p cb (h w)", p=P)
        w_bf = wpool.tile([P, KS * KS * OB * CB * P], BF16, name="w_bf")
        nc.sync.dma_start(w_bf[:, 0:WSPLIT], wT_ap[:, 0:WSPLIT])
        # image 0's first row-chunk rides between the two weight halves so the
        # first sign lands as early as possible
        xA0 = xpool.tile([P, CB, RA * H], BF16, name="xA", tag="xA")
        nc.sync.dma_start(xA0, x_v[0][:, :, 0 : RA * H])
        nc.sync.dma_start(
            w_bf[:, WSPLIT : KS * KS * OB * CB * P],
            wT_ap[:, WSPLIT : KS * KS * OB * CB * P],
        )
        wT2 = wpool.tile([P, KS * KS * OB * CB * P], FP8, name="wT2")
        nc.scalar.sign(wT2[:, 0:WSPLIT], w_bf[:, 0:WSPLIT])
        # ob=1 half's sign is emitted after image 0's row-chunk signs
        # wv[i_low, ob, tap, cb, o]; pair dim cb has byte-step 128 (%16==0)
        wv = wT2.rearrange("p (ob kk cb o) -> p ob kk cb o", kk=KS * KS, ob=OB, cb=CB)

        # ---- PE warm-up fillers: keep HAM at 2.4 GHz until the conv stream ----
        ones_t = const_pool.tile([1, NT], BF16, name="ones_t")
        nc.gpsimd.memset(ones_t, 1.0)
        warm_ps = psumpool.tile([P, NT], F32, name="warm", tag="warm", bufs=1)
        for _ in range(N_WARM):
            nc.tensor.matmul(
                warm_ps, ones_t[0:1, 0:P], ones_t, start=True, stop=True
            )

        # ---- persistent padded sign images; borders zeroed once ----
        ims = [
            impool.tile([P, CB, HP, WP], FP8, name=f"im{i}", tag=f"im{i}")
            for i in range(2)
        ]
        for im in ims:
            nc.gpsimd.memset(im[:, :, 0, 0:58], 0.0)
            nc.gpsimd.memset(im[:, :, HP - 1, 0:58], 0.0)
            nc.gpsimd.memset(im[:, :, 1 : HP - 1, 0], 0.0)
            nc.gpsimd.memset(im[:, :, 1 : HP - 1, 57], 0.0)

        abg = const_pool.tile([P, OB, HW], BF16, name="abg")

        # ---- main loop over local batches ----
        out_v = out_ap.rearrange("b (ob p) h w -> b ob p (h w)", p=P)
        for b in range(BL):
            im = ims[b % 2]
            if b == 0:
                xA = xA0
            else:
                xA = xpool.tile([P, CB, RA * H], BF16, name="xA", tag="xA")
                nc.sync.dma_start(xA, x_v[b][:, :, 0 : RA * H])
            xT = xpool.tile([P, CB, RT * H], BF16, name="xT", tag="xT")
            nc.sync.dma_start(xT, x_v[b][:, :, RA * H : (RA + RT) * H])
            xB = xpool.tile([P, CB, RB * H], BF16, name="xB", tag="xB")
            nc.sync.dma_start(xB, x_v[b][:, :, (RA + RT) * H : HW])
            views = [
                (0, xA.rearrange("p c (h w) -> p c h w", h=RA)),
                (RA, xT.rearrange("p c (h w) -> p c h w", h=RT)),
                (RA + RT, xB.rearrange("p c (h w) -> p c h w", h=RB)),
            ]
            for r0, r1 in CHUNKS:
                v0, view = next(v for v in reversed(views) if v[0] <= r0)
                nc.scalar.sign(
                    im[:, :, 1 + r0 : 1 + r1, 1 : 1 + H],
                    view[:, :, r0 - v0 : r1 - v0, :],
                )
            if b == 0:
                nc.scalar.sign(
                    wT2[:, WSPLIT : KS * KS * OB * CB * P],
                    w_bf[:, WSPLIT : KS * KS * OB * CB * P],
                )

            if b == 0:
                # tiny scale DMAs + alpha*beta*gamma map. Emitted after the
                # startup-critical dispatches (w, image-0 x) so the Sync engine
                # doesn't delay those. The map must be ready ~12us after the
                # first conv MM (when ob=0's epilogues must free PSUM banks),
                # so it is built from two tiny [1,56] partition-broadcasts and
                # DVE outer-product muls -- a ~5us chain, all small ops.
                a_t = const_pool.tile([P, OB], F32, name="a_t")
                nc.sync.dma_start(
                    a_t, a_ap.rearrange("(ob p) u v -> p (ob u v)", p=P)
                )
                b_t = const_pool.tile([1, H], F32, name="b_t")
                nc.sync.dma_start(b_t, b_ap[0:1, :, 0])
                g_t = const_pool.tile([1, H], F32, name="g_t")
                nc.sync.dma_start(g_t, g_ap[0:1, 0, :])
                b_bc = const_pool.tile([P, H], F32, name="b_bc")
                nc.gpsimd.partition_broadcast(b_bc, b_t)
                g_bc = const_pool.tile([P, H], F32, name="g_bc")
                nc.gpsimd.partition_broadcast(g_bc, g_t)
                ab = const_pool.tile([P, OB, H], F32, name="ab")
                for ob in range(OB):
                    nc.vector.tensor_scalar_mul(
                        ab[:, ob, :], b_bc, a_t[:, ob : ob + 1]
                    )
                    nc.vector.tensor_mul(
                        abg[:, ob, :].rearrange("p (i j) -> p i j", i=H),
                        ab[:, ob, :].unsqueeze(2).to_broadcast((P, H, H)),
                        g_bc.unsqueeze(1).to_broadcast((P, H, H)),
                    )

            for ob in range(OB):
                for t in range(T):
                    ps = psumpool.tile([P, NT], F32, name="cps", tag="cps", bufs=7)
                    for kk in range(KS * KS):
                        ky, kx = divmod(kk, KS)
                        rhs = im[:, :, t * R + ky : t * R + ky + R, kx : kx + H]
                        nc.tensor.matmul(
                            ps,
                            wv[:, ob, kk],
                            rhs,
                            start=(kk == 0),
                            stop=(kk == KS * KS - 1),
                            perf_mode=DR,
                        )
                    sl = slice(t * NT, (t + 1) * NT)
                    ot = opool.tile([P, NT], BF16, name="ot")
                    if b == 0 and ob == 0:
                        # abg may not be ready yet -- copy PSUM out first so
                        # the bank frees for ob=1's matmuls, multiply later
                        tmp = opool.tile([P, NT], F32, name="etmp", tag="etmp")
                        nc.vector.tensor_copy(tmp, ps)
                        nc.vector.tensor_mul(ot, tmp, abg[:, ob, sl])
                    else:
                        nc.vector.tensor_mul(ot, ps, abg[:, ob, sl])
                    nc.sync.dma_start(out_v[b, ob][:, sl], ot)


def build_nc(BL):
    nc = bacc.Bacc("TRN2", target_bir_lowering=False, debug=False)
    x = nc.dram_tensor("x", [BL, C, H, H], BF16, kind="ExternalInput")
    wT = nc.dram_tensor(
        "weightT", [P, KS * KS * OB * CB * P], BF16, kind="ExternalInput"
    )
    a = nc.dram_tensor("alpha", [C, 1, 1], F32, kind="ExternalInput")
    be = nc.dram_tensor("beta", [1, H, 1], F32, kind="ExternalInput")
    g = nc.dram_tensor("gamma", [1, 1, H], F32, kind="ExternalInput")
    o = nc.dram_tensor("out", [BL, C, H, H], BF16, kind="ExternalOutput")
    with tile.TileContext(nc) as tc:
        build_conv(tc, o.ap(), x.ap(), wT.ap(), a.ap(), be.ap(), g.ap(), BL)
    nc.compile()
    return nc


_nc_cache = {}


def _get_nc(BL):
    if BL not in _nc_cache:
        _nc_cache[BL] = build_nc(BL)
    return _nc_cache[BL]


def _prep(x, weight, alpha, beta, gamma):
    """Build the bass kernel and the per-core input maps."""
    x = np.asarray(x, dtype=np.float32)
    weight = np.asarray(weight, dtype=np.float32)
    alpha = np.ascontiguousarray(np.asarray(alpha, dtype=np.float32))
    beta = np.ascontiguousarray(np.asarray(beta, dtype=np.float32))
    gamma = np.ascontiguousarray(np.asarray(gamma, dtype=np.float32))

    # bf16 staging: sign(bf16(v)) == sign(v) for all practically occurring values
    x_bf = np.ascontiguousarray(x.astype(ml_dtypes.bfloat16))
    # [o, i, ky, kx] -> [i_low, ob, (ky kx), cb, o_low]
    w6 = weight.reshape(OB, P, CB, P, KS, KS)
    wT = np.ascontiguousarray(
        w6.transpose(3, 0, 4, 5, 2, 1).astype(ml_dtypes.bfloat16)
    ).reshape(P, KS * KS * OB * CB * P)

    BL = B // N_CORES
    nc = _get_nc(BL)
    xs = x_bf.reshape(N_CORES, BL, C, H, H)
    in_maps = [
        {"x": xs[c], "weightT": wT, "alpha": alpha, "beta": beta, "gamma": gamma}
        for c in range(N_CORES)
    ]
    return nc, in_maps


def kernel(x, weight, alpha, beta, gamma):
    nc, in_maps = _prep(x, weight, alpha, beta, gamma)
    res = run_bass_kernel_spmd(nc, in_maps, list(range(N_CORES)))
    out = np.concatenate([r["out"] for r in res.results], axis=0)
    return out.astype(np.float32)
